# revision 13
# baseline (speedup 1.0000x reference)
"""Causal single-head attention (B=4, S=4096, D=128, fp32 I/O) on 8 TRN2 cores.

Sharding: core = (batch b, parity c); core 2*b+c computes queries q ≡ c (mod 2)
of batch b.  The mod-2 query interleave makes every core's causal workload
structurally identical (SPMD: one graph, per-core data), and balances the
causal triangle exactly.

Device algorithm per core (all host tensors pre-transposed on host):
  - projections: K^T = W_K^T @ embT chunks, Q^T likewise, V natural; K/Q/V
    stored bf16 in SBUF, fp32 PSUM accumulation.
  - scores computed transposed (keys on PSUM partitions): S^T = KT_tile.T @ QT,
    packed variable-width (only causally needed query columns), one Exp
    activation per 3-k-tile chunk, fp32->bf16.
  - causal diagonal handled by multiplying the two per-parity 128x128 masks.
  - P@V via lhsT=P^T chunks, rhs=V augmented with a ones column: PSUM
    accumulates both the weighted values and the softmax denominator.
  - normalize by reciprocal of the ones column, DMA out fp32.
"""

import math

import ml_dtypes
import numpy as np

B, S, D = 4, 4096, 128
SQ = S // 2          # queries per core
N_KT = S // 128      # 32 key tiles
N_J = SQ // 512      # 4 query super-tiles of 512
SCALE = 1.0 / math.sqrt(D)
CHUNK = 3            # k-tiles per exp chunk (3 psum banks, double buffered)

_GRAPH_CACHE = {}


def _patch_drain_split():
    """This container's walrus rejects >2 sync-waits on one Drain instruction
    (CoreV3GenImpl setupSyncWait: "Too many sync wait commands").  Split the
    TileContext exit-drain's waits across separate nop instructions."""
    import concourse.tile as tile
    from concourse.vector_clock import ScopedClock

    if getattr(tile.TileContext, "_drain_split_patched", False):
        return

    def _drain_and_barrier(self, tick_clock, wait_clock):
        nc = self.nc
        drain_inst = nc.sync.drain()
        wait_clock.add_sem_waits(
            drain_inst.ins, ScopedClock({None: tick_clock.global_clock})
        )
        waits = list(drain_inst.ins.sync_info.on_wait)
        maxw = 1
        if len(waits) > maxw:
            drain_inst.ins.sync_info.on_wait = waits[:maxw]
            import concourse.mybir as mybir
            for i in range(maxw, len(waits), maxw):
                nop = nc.sync.nop(nofuse=True, hint="drain_wait_split")
                nop.ins.sync_info = mybir.SyncInfo(
                    on_wait=waits[i:i + maxw], on_update=[]
                )
        nc.all_engine_barrier()
        assert self.sems is not None
        popped = nc._tile_sem_poison_stack.pop()
        assert popped is self._sem_poison
        nc.clear_and_free_semaphores(list(self.sems.allocated().values()))
        nc.all_engine_barrier()

    tile.TileContext._drain_and_barrier = _drain_and_barrier
    tile.TileContext._drain_split_patched = True


def _split_excess_waits(nc, max_waits=1):
    """This container's walrus rejects instructions carrying more than a
    couple of sync-waits ("Too many sync wait commands").  Hoist excess
    waits onto same-engine nop instructions inserted just before."""
    import concourse.mybir as mybir

    for f in nc.m.functions:
        for bb in f.blocks:
            insts = list(bb.instructions)
            out = []
            changed = False
            for inst in insts:
                si = inst.sync_info
                if si is not None and si.on_wait and len(si.on_wait) > max_waits:
                    waits = list(si.on_wait)
                    for i in range(0, len(waits) - max_waits, max_waits):
                        nop = mybir.InstNoOp(
                            name=nc.get_next_instruction_name(),
                            engine=inst.engine,
                            bass_nofuse=True,
                            sync_info=mybir.SyncInfo(
                                on_wait=waits[i:i + max_waits], on_update=[]
                            ),
                        )
                        nc.register_instruction(nop, overwrite=True)
                        out.append(nop)
                    kept = waits[len(waits) - max_waits:]
                    inst.sync_info = mybir.SyncInfo(
                        on_wait=kept, on_update=list(si.on_update)
                    )
                    changed = True
                out.append(inst)
            if changed:
                bb.instructions = out


def _build_graph():
    import concourse.bass as bass
    import concourse.mybir as mybir
    import concourse.tile as tile

    _patch_drain_split()

    f32 = mybir.dt.float32
    bf16 = mybir.dt.bfloat16

    nc = bass.Bass()
    embT_d = nc.declare_dram_parameter("embT", [128, S], bf16, isOutput=False)
    embqT_d = nc.declare_dram_parameter("embqT", [128, SQ], bf16, isOutput=False)
    wq_d = nc.declare_dram_parameter("wq", [128, 128], bf16, isOutput=False)
    wk_d = nc.declare_dram_parameter("wk", [128, 128], bf16, isOutput=False)
    wv_d = nc.declare_dram_parameter("wv", [128, 128], bf16, isOutput=False)
    mask_d = nc.declare_dram_parameter("mask", [2, 128, 128], bf16, isOutput=False)
    out_d = nc.declare_dram_parameter("out", [SQ, 128], f32, isOutput=True)

    with tile.TileContext(nc) as tc:
        with (
            tc.tile_pool(name="const", bufs=1) as const,
            tc.tile_pool(name="pt", bufs=3) as pt_pool,
            tc.tile_pool(name="outp", bufs=4) as outp,
            tc.tile_pool(name="spsum", bufs=2, space="PSUM") as spsum,
            tc.tile_pool(name="pvpsum", bufs=1, space="PSUM") as pvpsum,
        ):
            # ---- loads ----
            embT_sb = const.tile([128, S], bf16)
            nc.sync.dma_start(out=embT_sb, in_=embT_d[:, :])
            embqT_sb = const.tile([128, SQ], bf16)
            nc.gpsimd.dma_start(out=embqT_sb, in_=embqT_d[:, :])
            wq_sb = const.tile([128, 128], bf16)
            nc.gpsimd.dma_start(out=wq_sb, in_=wq_d[:, :])
            wk_sb = const.tile([128, 128], bf16)
            nc.gpsimd.dma_start(out=wk_sb, in_=wk_d[:, :])
            wv_sb = const.tile([128, 128], bf16)
            nc.gpsimd.dma_start(out=wv_sb, in_=wv_d[:, :])
            mask_tile = const.tile([128, 2, 128], bf16)
            for t in range(2):
                nc.gpsimd.dma_start(out=mask_tile[:, t, :], in_=mask_d[t])
            mask_sb = [mask_tile[:, 0, :], mask_tile[:, 1, :]]

            # warm the ACT exp table-set (~2.7us load) under the embT DMA
            warm = const.tile([128, 1], mybir.dt.float32)
            nc.scalar.activation(
                warm, wq_sb[:, 0:1], mybir.ActivationFunctionType.Exp,
                scale=SCALE,
            )

            # ---- projections ----
            KT_sb = const.tile([128, S], bf16)
            QT_sb = const.tile([128, SQ], bf16)
            Vaug_sb = const.tile([128, N_KT, 129], bf16)
            nc.vector.memset(Vaug_sb[:, :, 128], 1.0)

            for i in range(S // 512):
                ps = spsum.tile([128, 1536], mybir.dt.float32, tag="sps")
                nc.tensor.matmul(
                    ps[:, 0:512], wk_sb, embT_sb[:, i * 512:(i + 1) * 512],
                    start=True, stop=True,
                )
                nc.scalar.copy(KT_sb[:, i * 512:(i + 1) * 512], ps[:, 0:512])
            for i in range(SQ // 512):
                ps = spsum.tile([128, 1536], mybir.dt.float32, tag="sps")
                nc.tensor.matmul(
                    ps[:, 0:512], wq_sb, embqT_sb[:, i * 512:(i + 1) * 512],
                    start=True, stop=True,
                )
                nc.scalar.copy(QT_sb[:, i * 512:(i + 1) * 512], ps[:, 0:512])
            for t in range(N_KT):
                ps = spsum.tile([128, 1536], mybir.dt.float32, tag="sps")
                nc.tensor.matmul(
                    ps[:, 0:128], embT_sb[:, t * 128:(t + 1) * 128], wv_sb,
                    start=True, stop=True,
                )
                nc.vector.tensor_copy(Vaug_sb[:, t, 0:128], ps[:, 0:128])

            # ---- attention ----
            for J in range(N_J):
                pv = pvpsum.tile([128, 1024], mybir.dt.float32)
                nkt = 8 * J + 8
                kts = list(range(nkt))
                for c0 in range(0, nkt, CHUNK):
                    chunk = kts[c0:c0 + CHUNK]
                    offs = []
                    off = 0
                    for kt in chunk:
                        t = kt - 8 * J
                        qmin = t // 2 if t >= 0 else 0
                        w = 128 * (4 - qmin)
                        offs.append((kt, qmin, w, off))
                        off += w
                    cw = off

                    ps = spsum.tile([128, 1536], mybir.dt.float32, tag="sps")
                    for kt, qmin, w, o in offs:
                        # split at PSUM bank boundaries (512 fp32 per bank)
                        po = o
                        while po < o + w:
                            pw = min(o + w - po, (po // 512 + 1) * 512 - po)
                            q0 = 512 * J + 128 * qmin + (po - o)
                            nc.tensor.matmul(
                                ps[:, po:po + pw],
                                KT_sb[:, kt * 128:(kt + 1) * 128],
                                QT_sb[:, q0:q0 + pw],
                                start=True, stop=True,
                            )
                            po += pw
                    pt = pt_pool.tile([128, 1536], mybir.dt.bfloat16, tag="pt")
                    nc.scalar.activation(
                        pt[:, 0:cw], ps[:, 0:cw],
                        mybir.ActivationFunctionType.Exp, scale=SCALE,
                    )
                    for kt, qmin, w, o in offs:
                        t = kt - 8 * J
                        if t >= 0:
                            q = t // 2
                            col = o + (q - qmin) * 128
                            nc.vector.tensor_mul(
                                pt[:, col:col + 128], pt[:, col:col + 128],
                                mask_sb[t % 2],
                            )
                        for q in range(qmin, 4):
                            col = o + (q - qmin) * 128
                            # accumulators are packed two per PSUM bank
                            # (q0,q1 -> bank0; q2,q3 -> bank1); one
                            # accumulation group per bank: start on the
                            # bank's first matmul, stop on its last.
                            nc.tensor.matmul(
                                pv[:, 256 * q:256 * q + 129],
                                pt[:, col:col + 128],
                                Vaug_sb[:, kt, :],
                                start=(kt == 0 and q % 2 == 0),
                                stop=(q % 2 == 1 and kt == 8 * J + 2 * q + 1),
                            )
                # normalize + store
                for q in range(4):
                    rc = outp.tile([128, 1], mybir.dt.float32, tag="rc")
                    nc.vector.reciprocal(rc, pv[:, 256 * q + 128:256 * q + 129])
                    ob = outp.tile([128, 128], mybir.dt.float32, tag="ob")
                    nc.vector.tensor_scalar_mul(ob, pv[:, 256 * q:256 * q + 128], rc)
                    r0 = 128 * (4 * J + q)
                    nc.sync.dma_start(out=out_d[r0:r0 + 128, :], in_=ob)
    _split_excess_waits(nc)
    return nc


def _get_graph():
    if "nc" not in _GRAPH_CACHE:
        _GRAPH_CACHE["nc"] = _build_graph()
    return _GRAPH_CACHE["nc"]


def _masks(c):
    kl = np.arange(128)[:, None]
    ql = np.arange(128)[None, :]
    m = np.empty((2, 128, 128), dtype=ml_dtypes.bfloat16)
    for t in range(2):
        m[t] = ((128 * t + kl) <= (2 * ql + c)).astype(ml_dtypes.bfloat16)
    return m


def _shard_inputs(embeddings, W_Q, W_K, W_V):
    emb = np.asarray(embeddings, dtype=np.float32)
    wq = np.ascontiguousarray(np.asarray(W_Q, dtype=np.float32)).astype(ml_dtypes.bfloat16)
    wk = np.ascontiguousarray(np.asarray(W_K, dtype=np.float32)).astype(ml_dtypes.bfloat16)
    wv = np.ascontiguousarray(np.asarray(W_V, dtype=np.float32)).astype(ml_dtypes.bfloat16)
    in_maps = []
    for core in range(8):
        b, c = divmod(core, 2)
        in_maps.append({
            "embT": np.ascontiguousarray(emb[b].T).astype(ml_dtypes.bfloat16),
            "embqT": np.ascontiguousarray(emb[b, c::2, :].T).astype(ml_dtypes.bfloat16),
            "wq": wq, "wk": wk, "wv": wv,
            "mask": _masks(c),
        })
    return in_maps


def _execute(in_maps, trace=False):
    from concourse.bass_utils import run_bass_kernel_spmd

    nc = _get_graph()
    res = run_bass_kernel_spmd(nc, in_maps, core_ids=list(range(8)), trace=trace)
    return res


def kernel(embeddings, W_Q, W_K, W_V):
    in_maps = _shard_inputs(embeddings, W_Q, W_K, W_V)
    res = _execute(in_maps, trace=False)
    out = np.empty((B, S, D), dtype=np.float32)
    for core in range(8):
        b, c = divmod(core, 2)
        out[b, c::2, :] = res.results[core]["out"]
    return out


# revision 14
# speedup vs baseline: 1.0126x; 1.0126x over previous
"""Causal single-head attention (B=4, S=4096, D=128, fp32 I/O) on 8 TRN2 cores.

Sharding: core = (batch b, parity c); core 2*b+c computes queries q ≡ c (mod 2)
of batch b.  The mod-2 query interleave makes every core's causal workload
structurally identical (SPMD: one graph, per-core data), and balances the
causal triangle exactly.

Device algorithm per core (all host tensors pre-transposed on host):
  - projections: K^T = W_K^T @ embT chunks, Q^T likewise, V natural; K/Q/V
    stored bf16 in SBUF, fp32 PSUM accumulation.
  - scores computed transposed (keys on PSUM partitions): S^T = KT_tile.T @ QT,
    packed variable-width (only causally needed query columns), one Exp
    activation per 3-k-tile chunk, fp32->bf16.
  - causal diagonal handled by multiplying the two per-parity 128x128 masks.
  - P@V via lhsT=P^T chunks, rhs=V augmented with a ones column: PSUM
    accumulates both the weighted values and the softmax denominator.
  - normalize by reciprocal of the ones column, DMA out fp32.
"""

import math

import ml_dtypes
import numpy as np

B, S, D = 4, 4096, 128
SQ = S // 2          # queries per core
N_KT = S // 128      # 32 key tiles
N_J = SQ // 512      # 4 query super-tiles of 512
SCALE = 1.0 / math.sqrt(D)
CHUNK = 3            # k-tiles per exp chunk (3 psum banks, double buffered)

_GRAPH_CACHE = {}


def _patch_drain_split():
    """This container's walrus rejects >2 sync-waits on one Drain instruction
    (CoreV3GenImpl setupSyncWait: "Too many sync wait commands").  Split the
    TileContext exit-drain's waits across separate nop instructions."""
    import concourse.tile as tile
    from concourse.vector_clock import ScopedClock

    if getattr(tile.TileContext, "_drain_split_patched", False):
        return

    def _drain_and_barrier(self, tick_clock, wait_clock):
        nc = self.nc
        drain_inst = nc.sync.drain()
        wait_clock.add_sem_waits(
            drain_inst.ins, ScopedClock({None: tick_clock.global_clock})
        )
        waits = list(drain_inst.ins.sync_info.on_wait)
        maxw = 1
        if len(waits) > maxw:
            drain_inst.ins.sync_info.on_wait = waits[:maxw]
            import concourse.mybir as mybir
            for i in range(maxw, len(waits), maxw):
                nop = nc.sync.nop(nofuse=True, hint="drain_wait_split")
                nop.ins.sync_info = mybir.SyncInfo(
                    on_wait=waits[i:i + maxw], on_update=[]
                )
        nc.all_engine_barrier()
        assert self.sems is not None
        popped = nc._tile_sem_poison_stack.pop()
        assert popped is self._sem_poison
        nc.clear_and_free_semaphores(list(self.sems.allocated().values()))
        nc.all_engine_barrier()

    tile.TileContext._drain_and_barrier = _drain_and_barrier
    tile.TileContext._drain_split_patched = True


def _split_excess_waits(nc, max_waits=1):
    """This container's walrus rejects instructions carrying more than a
    couple of sync-waits ("Too many sync wait commands").  Hoist excess
    waits onto same-engine nop instructions inserted just before."""
    import concourse.mybir as mybir

    for f in nc.m.functions:
        for bb in f.blocks:
            insts = list(bb.instructions)
            out = []
            changed = False
            for inst in insts:
                si = inst.sync_info
                if si is not None and si.on_wait and len(si.on_wait) > max_waits:
                    waits = list(si.on_wait)
                    for i in range(0, len(waits) - max_waits, max_waits):
                        nop = mybir.InstNoOp(
                            name=nc.get_next_instruction_name(),
                            engine=inst.engine,
                            bass_nofuse=True,
                            sync_info=mybir.SyncInfo(
                                on_wait=waits[i:i + max_waits], on_update=[]
                            ),
                        )
                        nc.register_instruction(nop, overwrite=True)
                        out.append(nop)
                    kept = waits[len(waits) - max_waits:]
                    inst.sync_info = mybir.SyncInfo(
                        on_wait=kept, on_update=list(si.on_update)
                    )
                    changed = True
                out.append(inst)
            if changed:
                bb.instructions = out


def _build_graph():
    import concourse.bass as bass
    import concourse.mybir as mybir
    import concourse.tile as tile

    _patch_drain_split()

    f32 = mybir.dt.float32
    bf16 = mybir.dt.bfloat16

    nc = bass.Bass()
    embT_d = nc.declare_dram_parameter("embT", [128, S], bf16, isOutput=False)
    embqT_d = nc.declare_dram_parameter("embqT", [128, SQ], bf16, isOutput=False)
    wq_d = nc.declare_dram_parameter("wq", [128, 128], bf16, isOutput=False)
    wk_d = nc.declare_dram_parameter("wk", [128, 128], bf16, isOutput=False)
    wv_d = nc.declare_dram_parameter("wv", [128, 128], bf16, isOutput=False)
    mask_d = nc.declare_dram_parameter("mask", [2, 128, 128], bf16, isOutput=False)
    out_d = nc.declare_dram_parameter("out", [SQ, 128], f32, isOutput=True)

    with tile.TileContext(nc) as tc:
        with (
            tc.tile_pool(name="const", bufs=1) as const,
            tc.tile_pool(name="pt", bufs=3) as pt_pool,
            tc.tile_pool(name="outp", bufs=4) as outp,
            tc.tile_pool(name="spsum", bufs=2, space="PSUM") as spsum,
            tc.tile_pool(name="pvpsum", bufs=1, space="PSUM") as pvpsum,
        ):
            # ---- loads ----
            embT_sb = const.tile([128, S], bf16)
            nc.sync.dma_start(out=embT_sb, in_=embT_d[:, :])
            embqT_sb = const.tile([128, SQ], bf16)
            nc.sync.dma_start(out=embqT_sb, in_=embqT_d[:, :])
            wq_sb = const.tile([128, 128], bf16)
            nc.sync.dma_start(out=wq_sb, in_=wq_d[:, :])
            wk_sb = const.tile([128, 128], bf16)
            nc.sync.dma_start(out=wk_sb, in_=wk_d[:, :])
            wv_sb = const.tile([128, 128], bf16)
            nc.sync.dma_start(out=wv_sb, in_=wv_d[:, :])
            mask_tile = const.tile([128, 2, 128], bf16)
            for t in range(2):
                nc.sync.dma_start(out=mask_tile[:, t, :], in_=mask_d[t])
            mask_sb = [mask_tile[:, 0, :], mask_tile[:, 1, :]]

            # warm the ACT exp table-set (~2.7us load) under the embT DMA
            warm = const.tile([128, 1], mybir.dt.float32)
            nc.scalar.activation(
                warm, wq_sb[:, 0:1], mybir.ActivationFunctionType.Exp,
                scale=SCALE,
            )

            # ---- projections ----
            KT_sb = const.tile([128, S], bf16)
            QT_sb = const.tile([128, SQ], bf16)
            Vaug_sb = const.tile([128, N_KT, 129], bf16)
            nc.vector.memset(Vaug_sb[:, :, 128], 1.0)

            for i in range(S // 512):
                ps = spsum.tile([128, 1536], mybir.dt.float32, tag="sps")
                nc.tensor.matmul(
                    ps[:, 0:512], wk_sb, embT_sb[:, i * 512:(i + 1) * 512],
                    start=True, stop=True,
                )
                nc.scalar.copy(KT_sb[:, i * 512:(i + 1) * 512], ps[:, 0:512])
            for i in range(SQ // 512):
                ps = spsum.tile([128, 1536], mybir.dt.float32, tag="sps")
                nc.tensor.matmul(
                    ps[:, 0:512], wq_sb, embqT_sb[:, i * 512:(i + 1) * 512],
                    start=True, stop=True,
                )
                nc.scalar.copy(QT_sb[:, i * 512:(i + 1) * 512], ps[:, 0:512])
            for t in range(N_KT):
                ps = spsum.tile([128, 1536], mybir.dt.float32, tag="sps")
                nc.tensor.matmul(
                    ps[:, 0:128], embT_sb[:, t * 128:(t + 1) * 128], wv_sb,
                    start=True, stop=True,
                )
                nc.vector.tensor_copy(Vaug_sb[:, t, 0:128], ps[:, 0:128])

            # ---- attention ----
            for J in range(N_J):
                pv = pvpsum.tile([128, 1024], mybir.dt.float32)
                nkt = 8 * J + 8
                kts = list(range(nkt))
                for c0 in range(0, nkt, CHUNK):
                    chunk = kts[c0:c0 + CHUNK]
                    offs = []
                    off = 0
                    for kt in chunk:
                        t = kt - 8 * J
                        qmin = t // 2 if t >= 0 else 0
                        w = 128 * (4 - qmin)
                        offs.append((kt, qmin, w, off))
                        off += w
                    cw = off

                    ps = spsum.tile([128, 1536], mybir.dt.float32, tag="sps")
                    for kt, qmin, w, o in offs:
                        # split at PSUM bank boundaries (512 fp32 per bank)
                        po = o
                        while po < o + w:
                            pw = min(o + w - po, (po // 512 + 1) * 512 - po)
                            q0 = 512 * J + 128 * qmin + (po - o)
                            nc.tensor.matmul(
                                ps[:, po:po + pw],
                                KT_sb[:, kt * 128:(kt + 1) * 128],
                                QT_sb[:, q0:q0 + pw],
                                start=True, stop=True,
                            )
                            po += pw
                    pt = pt_pool.tile([128, 1536], mybir.dt.bfloat16, tag="pt")
                    nc.scalar.activation(
                        pt[:, 0:cw], ps[:, 0:cw],
                        mybir.ActivationFunctionType.Exp, scale=SCALE,
                    )
                    for kt, qmin, w, o in offs:
                        t = kt - 8 * J
                        if t >= 0:
                            q = t // 2
                            col = o + (q - qmin) * 128
                            nc.vector.tensor_mul(
                                pt[:, col:col + 128], pt[:, col:col + 128],
                                mask_sb[t % 2],
                            )
                        for q in range(qmin, 4):
                            col = o + (q - qmin) * 128
                            # accumulators are packed two per PSUM bank
                            # (q0,q1 -> bank0; q2,q3 -> bank1); one
                            # accumulation group per bank: start on the
                            # bank's first matmul, stop on its last.
                            nc.tensor.matmul(
                                pv[:, 256 * q:256 * q + 129],
                                pt[:, col:col + 128],
                                Vaug_sb[:, kt, :],
                                start=(kt == 0 and q % 2 == 0),
                                stop=(q % 2 == 1 and kt == 8 * J + 2 * q + 1),
                            )
                # normalize + store
                for q in range(4):
                    rc = outp.tile([128, 1], mybir.dt.float32, tag="rc")
                    nc.vector.reciprocal(rc, pv[:, 256 * q + 128:256 * q + 129])
                    ob = outp.tile([128, 128], mybir.dt.float32, tag="ob")
                    nc.vector.tensor_scalar_mul(ob, pv[:, 256 * q:256 * q + 128], rc)
                    r0 = 128 * (4 * J + q)
                    nc.sync.dma_start(out=out_d[r0:r0 + 128, :], in_=ob)
    _split_excess_waits(nc)
    return nc


def _get_graph():
    if "nc" not in _GRAPH_CACHE:
        _GRAPH_CACHE["nc"] = _build_graph()
    return _GRAPH_CACHE["nc"]


def _masks(c):
    kl = np.arange(128)[:, None]
    ql = np.arange(128)[None, :]
    m = np.empty((2, 128, 128), dtype=ml_dtypes.bfloat16)
    for t in range(2):
        m[t] = ((128 * t + kl) <= (2 * ql + c)).astype(ml_dtypes.bfloat16)
    return m


def _shard_inputs(embeddings, W_Q, W_K, W_V):
    emb = np.asarray(embeddings, dtype=np.float32)
    wq = np.ascontiguousarray(np.asarray(W_Q, dtype=np.float32)).astype(ml_dtypes.bfloat16)
    wk = np.ascontiguousarray(np.asarray(W_K, dtype=np.float32)).astype(ml_dtypes.bfloat16)
    wv = np.ascontiguousarray(np.asarray(W_V, dtype=np.float32)).astype(ml_dtypes.bfloat16)
    in_maps = []
    for core in range(8):
        b, c = divmod(core, 2)
        in_maps.append({
            "embT": np.ascontiguousarray(emb[b].T).astype(ml_dtypes.bfloat16),
            "embqT": np.ascontiguousarray(emb[b, c::2, :].T).astype(ml_dtypes.bfloat16),
            "wq": wq, "wk": wk, "wv": wv,
            "mask": _masks(c),
        })
    return in_maps


def _execute(in_maps, trace=False):
    from concourse.bass_utils import run_bass_kernel_spmd

    nc = _get_graph()
    res = run_bass_kernel_spmd(nc, in_maps, core_ids=list(range(8)), trace=trace)
    return res


def kernel(embeddings, W_Q, W_K, W_V):
    in_maps = _shard_inputs(embeddings, W_Q, W_K, W_V)
    res = _execute(in_maps, trace=False)
    out = np.empty((B, S, D), dtype=np.float32)
    for core in range(8):
        b, c = divmod(core, 2)
        out[b, c::2, :] = res.results[core]["out"]
    return out


# revision 15
# speedup vs baseline: 1.0229x; 1.0102x over previous
"""Causal single-head attention (B=4, S=4096, D=128, fp32 I/O) on 8 TRN2 cores.

Sharding: core = (batch b, parity c); core 2*b+c computes queries q ≡ c (mod 2)
of batch b.  The mod-2 query interleave makes every core's causal workload
structurally identical (SPMD: one graph, per-core data), and balances the
causal triangle exactly.

Device algorithm per core (all host tensors pre-transposed on host):
  - projections: K^T = W_K^T @ embT chunks, Q^T likewise, V natural; K/Q/V
    stored bf16 in SBUF, fp32 PSUM accumulation.
  - scores computed transposed (keys on PSUM partitions): S^T = KT_tile.T @ QT,
    packed variable-width (only causally needed query columns), one Exp
    activation per 3-k-tile chunk, fp32->bf16.
  - causal diagonal handled by multiplying the two per-parity 128x128 masks.
  - P@V via lhsT=P^T chunks, rhs=V augmented with a ones column: PSUM
    accumulates both the weighted values and the softmax denominator.
  - normalize by reciprocal of the ones column, DMA out fp32.
"""

import math

import ml_dtypes
import numpy as np

B, S, D = 4, 4096, 128
SQ = S // 2          # queries per core
N_KT = S // 128      # 32 key tiles
N_J = SQ // 512      # 4 query super-tiles of 512
SCALE = 1.0 / math.sqrt(D)
CHUNK = 3            # k-tiles per exp chunk (3 psum banks, double buffered)

_GRAPH_CACHE = {}


def _patch_drain_split():
    """This container's walrus rejects >2 sync-waits on one Drain instruction
    (CoreV3GenImpl setupSyncWait: "Too many sync wait commands").  Split the
    TileContext exit-drain's waits across separate nop instructions."""
    import concourse.tile as tile
    from concourse.vector_clock import ScopedClock

    if getattr(tile.TileContext, "_drain_split_patched", False):
        return

    def _drain_and_barrier(self, tick_clock, wait_clock):
        nc = self.nc
        drain_inst = nc.sync.drain()
        wait_clock.add_sem_waits(
            drain_inst.ins, ScopedClock({None: tick_clock.global_clock})
        )
        waits = list(drain_inst.ins.sync_info.on_wait)
        maxw = 1
        if len(waits) > maxw:
            drain_inst.ins.sync_info.on_wait = waits[:maxw]
            import concourse.mybir as mybir
            for i in range(maxw, len(waits), maxw):
                nop = nc.sync.nop(nofuse=True, hint="drain_wait_split")
                nop.ins.sync_info = mybir.SyncInfo(
                    on_wait=waits[i:i + maxw], on_update=[]
                )
        nc.all_engine_barrier()
        assert self.sems is not None
        popped = nc._tile_sem_poison_stack.pop()
        assert popped is self._sem_poison
        nc.clear_and_free_semaphores(list(self.sems.allocated().values()))
        nc.all_engine_barrier()

    tile.TileContext._drain_and_barrier = _drain_and_barrier
    tile.TileContext._drain_split_patched = True


def _split_excess_waits(nc, max_waits=1):
    """This container's walrus rejects instructions carrying more than a
    couple of sync-waits ("Too many sync wait commands").  Hoist excess
    waits onto same-engine nop instructions inserted just before."""
    import concourse.mybir as mybir

    for f in nc.m.functions:
        for bb in f.blocks:
            insts = list(bb.instructions)
            out = []
            changed = False
            for inst in insts:
                si = inst.sync_info
                if si is not None and si.on_wait and len(si.on_wait) > max_waits:
                    waits = list(si.on_wait)
                    for i in range(0, len(waits) - max_waits, max_waits):
                        nop = mybir.InstNoOp(
                            name=nc.get_next_instruction_name(),
                            engine=inst.engine,
                            bass_nofuse=True,
                            sync_info=mybir.SyncInfo(
                                on_wait=waits[i:i + max_waits], on_update=[]
                            ),
                        )
                        nc.register_instruction(nop, overwrite=True)
                        out.append(nop)
                    kept = waits[len(waits) - max_waits:]
                    inst.sync_info = mybir.SyncInfo(
                        on_wait=kept, on_update=list(si.on_update)
                    )
                    changed = True
                out.append(inst)
            if changed:
                bb.instructions = out


def _build_graph():
    import concourse.bass as bass
    import concourse.mybir as mybir
    import concourse.tile as tile

    _patch_drain_split()

    f32 = mybir.dt.float32
    bf16 = mybir.dt.bfloat16

    nc = bass.Bass()
    embT_d = nc.declare_dram_parameter("embT", [128, S], bf16, isOutput=False)
    embqT_d = nc.declare_dram_parameter("embqT", [128, SQ], bf16, isOutput=False)
    wq_d = nc.declare_dram_parameter("wq", [128, 128], bf16, isOutput=False)
    wk_d = nc.declare_dram_parameter("wk", [128, 128], bf16, isOutput=False)
    wv_d = nc.declare_dram_parameter("wv", [128, 128], bf16, isOutput=False)
    mask_d = nc.declare_dram_parameter("mask", [2, 128, 128], bf16, isOutput=False)
    out_d = nc.declare_dram_parameter("out", [SQ, 128], f32, isOutput=True)

    with tile.TileContext(nc) as tc:
        with (
            tc.tile_pool(name="const", bufs=1) as const,
            tc.tile_pool(name="pt", bufs=4) as pt_pool,
            tc.tile_pool(name="outp", bufs=6) as outp,
            tc.tile_pool(name="spsum", bufs=2, space="PSUM") as spsum,
            tc.tile_pool(name="pvpsum", bufs=1, space="PSUM") as pvpsum,
        ):
            # ---- loads ----
            embT_sb = const.tile([128, S], bf16)
            nc.sync.dma_start(out=embT_sb, in_=embT_d[:, :])
            embqT_sb = const.tile([128, SQ], bf16)
            nc.sync.dma_start(out=embqT_sb, in_=embqT_d[:, :])
            wq_sb = const.tile([128, 128], bf16)
            nc.sync.dma_start(out=wq_sb, in_=wq_d[:, :])
            wk_sb = const.tile([128, 128], bf16)
            nc.sync.dma_start(out=wk_sb, in_=wk_d[:, :])
            wv_sb = const.tile([128, 128], bf16)
            nc.sync.dma_start(out=wv_sb, in_=wv_d[:, :])
            mask_tile = const.tile([128, 2, 128], bf16)
            for t in range(2):
                nc.sync.dma_start(out=mask_tile[:, t, :], in_=mask_d[t])
            mask_sb = [mask_tile[:, 0, :], mask_tile[:, 1, :]]

            # warm the ACT exp table-set (~2.7us load) under the embT DMA
            warm = const.tile([128, 1], mybir.dt.float32)
            nc.scalar.activation(
                warm, wq_sb[:, 0:1], mybir.ActivationFunctionType.Exp,
                scale=SCALE,
            )

            # ---- projections ----
            KT_sb = const.tile([128, S], bf16)
            QT_sb = const.tile([128, SQ], bf16)
            Vaug_sb = const.tile([128, N_KT, 129], bf16)
            nc.vector.memset(Vaug_sb[:, :, 128], 1.0)

            for i in range(S // 512):
                ps = spsum.tile([128, 1536], mybir.dt.float32, tag="sps")
                nc.tensor.matmul(
                    ps[:, 0:512], wk_sb, embT_sb[:, i * 512:(i + 1) * 512],
                    start=True, stop=True,
                )
                nc.scalar.copy(KT_sb[:, i * 512:(i + 1) * 512], ps[:, 0:512])
            for i in range(SQ // 512):
                ps = spsum.tile([128, 1536], mybir.dt.float32, tag="sps")
                nc.tensor.matmul(
                    ps[:, 0:512], wq_sb, embqT_sb[:, i * 512:(i + 1) * 512],
                    start=True, stop=True,
                )
                nc.scalar.copy(QT_sb[:, i * 512:(i + 1) * 512], ps[:, 0:512])
            for t in range(N_KT):
                ps = spsum.tile([128, 1536], mybir.dt.float32, tag="sps")
                nc.tensor.matmul(
                    ps[:, 0:128], embT_sb[:, t * 128:(t + 1) * 128], wv_sb,
                    start=True, stop=True,
                )
                nc.vector.tensor_copy(Vaug_sb[:, t, 0:128], ps[:, 0:128])

            # ---- attention ----
            for J in range(N_J):
                pv = pvpsum.tile([128, 1024], mybir.dt.float32)
                nkt = 8 * J + 8
                kts = list(range(nkt))
                for c0 in range(0, nkt, CHUNK):
                    chunk = kts[c0:c0 + CHUNK]
                    offs = []
                    off = 0
                    for kt in chunk:
                        t = kt - 8 * J
                        qmin = t // 2 if t >= 0 else 0
                        w = 128 * (4 - qmin)
                        offs.append((kt, qmin, w, off))
                        off += w
                    cw = off

                    ps = spsum.tile([128, 1536], mybir.dt.float32, tag="sps")
                    for kt, qmin, w, o in offs:
                        # split at PSUM bank boundaries (512 fp32 per bank)
                        po = o
                        while po < o + w:
                            pw = min(o + w - po, (po // 512 + 1) * 512 - po)
                            q0 = 512 * J + 128 * qmin + (po - o)
                            nc.tensor.matmul(
                                ps[:, po:po + pw],
                                KT_sb[:, kt * 128:(kt + 1) * 128],
                                QT_sb[:, q0:q0 + pw],
                                start=True, stop=True,
                            )
                            po += pw
                    pt = pt_pool.tile([128, 1536], mybir.dt.bfloat16, tag="pt")
                    nc.scalar.activation(
                        pt[:, 0:cw], ps[:, 0:cw],
                        mybir.ActivationFunctionType.Exp, scale=SCALE,
                    )
                    for kt, qmin, w, o in offs:
                        t = kt - 8 * J
                        if t >= 0:
                            q = t // 2
                            col = o + (q - qmin) * 128
                            nc.vector.tensor_mul(
                                pt[:, col:col + 128], pt[:, col:col + 128],
                                mask_sb[t % 2],
                            )
                        for q in range(qmin, 4):
                            col = o + (q - qmin) * 128
                            # accumulators are packed two per PSUM bank
                            # (q0,q1 -> bank0; q2,q3 -> bank1); one
                            # accumulation group per bank: start on the
                            # bank's first matmul, stop on its last.
                            nc.tensor.matmul(
                                pv[:, 256 * q:256 * q + 129],
                                pt[:, col:col + 128],
                                Vaug_sb[:, kt, :],
                                start=(kt == 0 and q % 2 == 0),
                                stop=(q % 2 == 1 and kt == 8 * J + 2 * q + 1),
                            )
                # normalize + store
                for q in range(4):
                    rc = outp.tile([128, 1], mybir.dt.float32, tag="rc")
                    nc.vector.reciprocal(rc, pv[:, 256 * q + 128:256 * q + 129])
                    ob = outp.tile([128, 128], mybir.dt.float32, tag="ob")
                    nc.vector.tensor_scalar_mul(ob, pv[:, 256 * q:256 * q + 128], rc)
                    r0 = 128 * (4 * J + q)
                    nc.sync.dma_start(out=out_d[r0:r0 + 128, :], in_=ob)
    _split_excess_waits(nc)
    return nc


def _get_graph():
    if "nc" not in _GRAPH_CACHE:
        _GRAPH_CACHE["nc"] = _build_graph()
    return _GRAPH_CACHE["nc"]


def _masks(c):
    kl = np.arange(128)[:, None]
    ql = np.arange(128)[None, :]
    m = np.empty((2, 128, 128), dtype=ml_dtypes.bfloat16)
    for t in range(2):
        m[t] = ((128 * t + kl) <= (2 * ql + c)).astype(ml_dtypes.bfloat16)
    return m


def _shard_inputs(embeddings, W_Q, W_K, W_V):
    emb = np.asarray(embeddings, dtype=np.float32)
    wq = np.ascontiguousarray(np.asarray(W_Q, dtype=np.float32)).astype(ml_dtypes.bfloat16)
    wk = np.ascontiguousarray(np.asarray(W_K, dtype=np.float32)).astype(ml_dtypes.bfloat16)
    wv = np.ascontiguousarray(np.asarray(W_V, dtype=np.float32)).astype(ml_dtypes.bfloat16)
    in_maps = []
    for core in range(8):
        b, c = divmod(core, 2)
        in_maps.append({
            "embT": np.ascontiguousarray(emb[b].T).astype(ml_dtypes.bfloat16),
            "embqT": np.ascontiguousarray(emb[b, c::2, :].T).astype(ml_dtypes.bfloat16),
            "wq": wq, "wk": wk, "wv": wv,
            "mask": _masks(c),
        })
    return in_maps


def _execute(in_maps, trace=False):
    from concourse.bass_utils import run_bass_kernel_spmd

    nc = _get_graph()
    res = run_bass_kernel_spmd(nc, in_maps, core_ids=list(range(8)), trace=trace)
    return res


def kernel(embeddings, W_Q, W_K, W_V):
    in_maps = _shard_inputs(embeddings, W_Q, W_K, W_V)
    res = _execute(in_maps, trace=False)
    out = np.empty((B, S, D), dtype=np.float32)
    for core in range(8):
        b, c = divmod(core, 2)
        out[b, c::2, :] = res.results[core]["out"]
    return out
